# revision 9
# baseline (speedup 1.0000x reference)
"""DigitCapsules Trainium2 kernel (8-core batch-data-parallel).

Math (per sample b):
  caps[k,o,hw] = sum_c x[c,hw] conv_w[k,o,c] + conv_b[k,o]
  prim[k,p,d]  = caps[k, g//36, g%36],  g = p*8+d    (pure reindex)
  u[k,p,e]     = sum_d prim[k,p,d] W2[k,p,d,e]
  3 rounds of routing-by-agreement -> v[k,e]

Device mapping per core (256 samples = 2 tiles of 128 on partitions):
  * conv as 36x2 matmuls, stationary xT[c;b] per hw, rhs conv_w[c;ko]
  * caps evac to SBUF [b; k*576+g] bf16; PE-transpose 128-col chunks ->
    primT[(k,p,d); b] (chunk t == pair-group t because 576 = 72*8)
  * u-step: 45 block-diagonal matmuls primT_t @ W2blk_t -> u[b; k,p,e];
    an extra cheap rhs (W2 summed over p, x0.1) accumulates s1 = first
    routing iteration's weighted sum for free
  * routing on DVE/ACT: softmax over k, weighted sums as TT-mult +
    reduce, squash with NR-refined sqrt
"""

import sys

sys.path.insert(0, "/opt/trn_rl_repo")

import numpy as np
import ml_dtypes

import concourse.bass as bass
import concourse.mybir as mybir
from concourse import tile
from concourse.vector_clock import ScopedClock
from concourse.masks import make_identity
from concourse.bass_utils import run_bass_kernel_spmd

# ---------------------------------------------------------------- constants
K, O, C, H, W = 10, 16, 256, 6, 6
HW = H * W                      # 36
PD, E, P = 8, 16, 72            # prim dim, out dim, prims per k
PAIRS = K * P                   # 720
GROUPS = PAIRS // 16            # 45 groups of 16 (k,p) pairs
KPE = PAIRS * E                 # 11520
KE = K * E                      # 160
KO = K * O                      # 160
B_TOTAL, N_CORES = 2048, 8
B_CORE = B_TOTAL // N_CORES     # 256
TB = 128                        # batch tile (partitions)
NT = B_CORE // TB               # 2
EPS = 1e-8

F32 = mybir.dt.float32
BF16 = mybir.dt.bfloat16


# ------------------------------------------------- tile drain-limit patch
# The TileContext exit drain accumulates one sem wait per outstanding lane,
# but this walrus build rejects >2 sync waits on a CTRL-struct instruction.
# Spill the extra waits onto single-wait SP nops before the barrier.
def _patched_drain_and_barrier(self, tick_clock, wait_clock):
    nc = self.nc
    drain_inst = nc.sync.drain()
    wait_clock.add_sem_waits(
        drain_inst.ins, ScopedClock({None: tick_clock.global_clock})
    )
    ow = list(drain_inst.ins.sync_info.on_wait)
    if len(ow) > 1:
        drain_inst.ins.sync_info.on_wait = ow[:1]
        for w in ow[1:]:
            ni = nc.sync.nop()
            ni.ins.sync_info = mybir.SyncInfo(on_wait=[w], on_update=[])
    nc.all_engine_barrier()
    assert self.sems is not None
    popped = nc._tile_sem_poison_stack.pop()
    assert popped is self._sem_poison
    nc.clear_and_free_semaphores(list(self.sems.allocated().values()))
    nc.all_engine_barrier()


tile.TileContext._drain_and_barrier = _patched_drain_and_barrier

_MAXW = 1  # max sync waits this walrus accepts per instruction
_wsplit_n = [0]


def _split_excess_waits(nc):
    """Walrus rejects >_MAXW sync waits on one instruction. Hoist extras onto
    same-engine NoOps placed immediately before the overloaded instruction
    (waits execute earlier -> strictly more conservative, still correct)."""
    for f in nc.m.functions:
        for blk in f.blocks:
            insts = list(blk.instructions)
            out = []
            for inst in insts:
                si = inst.sync_info
                if si is not None and si.on_wait and len(si.on_wait) > _MAXW:
                    ow = list(si.on_wait)
                    extra, keep = ow[:-_MAXW], ow[-_MAXW:]
                    for i in range(0, len(extra), _MAXW):
                        _wsplit_n[0] += 1
                        out.append(
                            mybir.InstNoOp(
                                name=f"I-wsplit-{_wsplit_n[0]}",
                                engine=inst.engine,
                                sync_info=mybir.SyncInfo(
                                    on_wait=extra[i:i + _MAXW], on_update=[]
                                ),
                            )
                        )
                    si.on_wait = keep
                out.append(inst)
            if len(out) != len(insts):
                blk.instructions[:] = out


def _squash(nc, rp, s_ap, v_f32, v_bf16):
    """v = (|s|^2/(1+|s|^2)) * s / sqrt(|s|^2+eps); s_ap [128, K*E] f32."""
    t2 = rp.tile([TB, KE], F32, tag="sq_t2")
    nc.vector.tensor_mul(t2[:], s_ap, s_ap)
    ss = rp.tile([TB, K], F32, tag="sq_ss")
    nc.vector.reduce_sum(
        ss[:], t2[:].rearrange("b (k e) -> b k e", k=K), axis=mybir.AxisListType.X
    )
    ssq = rp.tile([TB, K], F32, tag="sq_ssq")
    nc.vector.tensor_scalar_add(ssq[:], ss[:], EPS)
    q = rp.tile([TB, K], F32, tag="sq_q")
    nc.scalar.sqrt(q[:], ssq[:])
    # one Newton step: q1 = 0.5*(q + ssq/q)  (guards ACT sqrt table error)
    rq = rp.tile([TB, K], F32, tag="sq_rq")
    nc.vector.reciprocal(rq[:], q[:])
    m1 = rp.tile([TB, K], F32, tag="sq_m1")
    nc.vector.tensor_mul(m1[:], ssq[:], rq[:])
    nc.vector.tensor_add(m1[:], m1[:], q[:])
    nc.vector.tensor_scalar_mul(m1[:], m1[:], 0.5)  # m1 = sqrt(ss+eps)
    a1 = rp.tile([TB, K], F32, tag="sq_a1")
    nc.vector.tensor_scalar_add(a1[:], ss[:], 1.0)
    nc.vector.tensor_mul(a1[:], a1[:], m1[:])       # (1+ss)*sqrt(ss+eps)
    rd = rp.tile([TB, K], F32, tag="sq_rd")
    nc.vector.reciprocal(rd[:], a1[:])
    f = rp.tile([TB, K], F32, tag="sq_f")
    nc.vector.tensor_mul(f[:], ss[:], rd[:])        # ss/((1+ss)*sqrt(..))
    fb = f[:, :, None].broadcast_to([TB, K, E])
    nc.vector.tensor_mul(
        v_f32.rearrange("b (k e) -> b k e", k=K), s_ap.rearrange("b (k e) -> b k e", k=K), fb
    )
    nc.vector.tensor_copy(v_bf16, v_f32)


def _build_nc(has_bias):
    nc = bass.Bass()
    x_d = nc.dram_tensor("xt", [2, 128, B_CORE * HW], F32, kind="ExternalInput")
    cw_d = nc.dram_tensor("cwt", [C, KO], F32, kind="ExternalInput")
    w2_d = nc.dram_tensor("w2blk", [GROUPS, 128, 256], BF16, kind="ExternalInput")
    w2s_d = nc.dram_tensor("w2s", [GROUPS, 128, KE], BF16, kind="ExternalInput")
    if has_bias:
        bu_d = nc.dram_tensor("biasu", [TB, KPE], BF16, kind="ExternalInput")
        bs_d = nc.dram_tensor("biass1", [TB, KE], F32, kind="ExternalInput")
    out_d = nc.dram_tensor("out", [B_CORE, KE], F32, kind="ExternalOutput")

    with tile.TileContext(nc) as tc:
        with (
            tc.tile_pool(name="consts", bufs=1) as consts,
            tc.tile_pool(name="xp", bufs=1) as xp,
            tc.tile_pool(name="big", bufs=1) as big,
            tc.tile_pool(name="rp", bufs=2) as rp,
            tc.tile_pool(name="ps_conv", bufs=2, space="PSUM") as ps_conv,
            tc.tile_pool(name="ps_tr", bufs=1, space="PSUM") as ps_tr,
            tc.tile_pool(name="ps_u", bufs=2, space="PSUM") as ps_u,
            tc.tile_pool(name="ps_s1", bufs=1, space="PSUM") as ps_s1p,
        ):
            cw0 = consts.tile([128, KO], F32)
            cw1 = consts.tile([128, KO], F32)
            nc.sync.dma_start(cw0[:], cw_d[0:128, :])
            nc.sync.dma_start(cw1[:], cw_d[128:256, :])
            w2t = consts.tile([128, GROUPS * 256], BF16)
            nc.sync.dma_start(
                w2t[:].rearrange("c (t n) -> c t n", t=GROUPS), w2_d[:].rearrange("t c n -> c t n")
            )
            w2st = consts.tile([128, GROUPS * KE], BF16)
            nc.sync.dma_start(
                w2st[:].rearrange("c (t n) -> c t n", t=GROUPS), w2s_d[:].rearrange("t c n -> c t n")
            )
            ident = consts.tile([128, 128], BF16)
            make_identity(nc, ident[:])
            if has_bias:
                but = consts.tile([TB, KPE], BF16)
                nc.sync.dma_start(but[:], bu_d[:])
                bst = consts.tile([TB, KE], F32)
                nc.sync.dma_start(bst[:], bs_d[:])

            for bt in range(NT):
                # ------------------------------------------------ x load
                xt0 = xp.tile([128, TB * HW], F32, tag="xt0")
                xt1 = xp.tile([128, TB * HW], F32, tag="xt1")
                nc.sync.dma_start(xt0[:], x_d[0, :, bt * TB * HW:(bt + 1) * TB * HW])
                nc.sync.dma_start(xt1[:], x_d[1, :, bt * TB * HW:(bt + 1) * TB * HW])
                x0v = xt0[:].rearrange("c (b hw) -> c b hw", hw=HW)
                x1v = xt1[:].rearrange("c (b hw) -> c b hw", hw=HW)

                # ------------------------------------------------ conv
                caps = big.tile([TB, K * O * HW], BF16, tag="caps")
                capsv = caps[:].rearrange("b (k o hw) -> b k o hw", k=K, o=O)
                for hb in range(HW // 3):  # 12 psum banks of 3 hw each
                    pc = ps_conv.tile([TB, 3 * KO], F32, tag="pconv")
                    for j in range(3):
                        hw = hb * 3 + j
                        nc.tensor.matmul(
                            pc[:, j * KO:(j + 1) * KO], x0v[:, :, hw], cw0[:],
                            start=True, stop=False,
                        )
                        nc.tensor.matmul(
                            pc[:, j * KO:(j + 1) * KO], x1v[:, :, hw], cw1[:],
                            start=False, stop=True,
                        )
                    # evac [b; j,k,o] -> caps[b; k,o,hw=hb*3+j]
                    nc.scalar.copy(
                        capsv[:, :, :, hb * 3:hb * 3 + 3],
                        pc[:].rearrange("b (j k o) -> b k o j", j=3, k=K),
                    )

                # ------------------------------------- transpose to primT
                primT = big.tile([128, GROUPS * 128], BF16, tag="primT")
                for tb4 in range(GROUPS // 5):  # 9 psum tiles of 5 chunks
                    pt = ps_tr.tile([128, 5 * 128], BF16, tag="ptr")
                    for j in range(5):
                        t = tb4 * 5 + j
                        nc.tensor.transpose(
                            pt[:, j * 128:(j + 1) * 128],
                            caps[:, t * 128:(t + 1) * 128],
                            ident[:],
                        )
                    nc.scalar.copy(
                        primT[:, tb4 * 5 * 128:(tb4 + 1) * 5 * 128], pt[:]
                    )

                # ------------------------------------------------ u-step
                u1 = big.tile([TB, KPE], BF16, tag="u1")
                ps1 = ps_s1p.tile([TB, KE], F32, tag="ps1")
                for ub in range(12):  # psum tiles of <=4 groups (2 banks)
                    ng = min(4, GROUPS - 4 * ub)
                    pu = ps_u.tile([TB, 4 * 256], F32, tag="pu")
                    for j in range(ng):
                        t = ub * 4 + j
                        nc.tensor.matmul(
                            pu[:, j * 256:(j + 1) * 256],
                            primT[:, t * 128:(t + 1) * 128],
                            w2t[:, t * 256:(t + 1) * 256],
                            start=True, stop=True,
                        )
                        nc.tensor.matmul(
                            ps1[:],
                            primT[:, t * 128:(t + 1) * 128],
                            w2st[:, t * KE:(t + 1) * KE],
                            start=(t == 0), stop=(t == GROUPS - 1),
                        )
                    nc.vector.tensor_copy(
                        u1[:, ub * 4 * 256:ub * 4 * 256 + ng * 256],
                        pu[:, :ng * 256],
                    )
                u1v = u1[:].rearrange("b (k p e) -> b k p e", k=K, p=P)
                if has_bias:
                    nc.vector.tensor_add(u1[:], u1[:], but[:])

                # ------------------------------------------------ routing
                s_sb = rp.tile([TB, KE], F32, tag="s_sb")
                nc.scalar.copy(s_sb[:], ps1[:])
                if has_bias:
                    nc.vector.tensor_add(s_sb[:], s_sb[:], bst[:])
                v32 = rp.tile([TB, KE], F32, tag="v32")
                vbf = rp.tile([TB, KE], BF16, tag="vbf")
                _squash(nc, rp, s_sb[:], v32[:], vbf[:])

                bl = rp.tile([TB, PAIRS], F32, tag="bl")
                tmp = big.tile([TB, KPE], BF16, tag="tmp")
                tmpv = tmp[:].rearrange("b (k p e) -> b k p e", k=K, p=P)

                for it in range(3):
                    if it > 0:
                        # softmax over k -> c, then s = sum_p c*u
                        eb = rp.tile([TB, PAIRS], BF16, tag="eb")
                        nc.scalar.activation(
                            eb[:], bl[:], mybir.ActivationFunctionType.Exp
                        )
                        z = rp.tile([TB, P], F32, tag="z")
                        nc.vector.reduce_sum(
                            z[:],
                            eb[:].rearrange("b (k p) -> b p k", k=K),
                            axis=mybir.AxisListType.X,
                        )
                        rz = rp.tile([TB, P], F32, tag="rz")
                        nc.vector.reciprocal(rz[:], z[:])
                        cbf = rp.tile([TB, PAIRS], BF16, tag="cbf")
                        nc.vector.tensor_mul(
                            cbf[:].rearrange("b (k p) -> b k p", k=K),
                            eb[:].rearrange("b (k p) -> b k p", k=K),
                            rz[:, None, :].broadcast_to([TB, K, P]),
                        )
                        nc.vector.tensor_mul(
                            tmpv,
                            u1v,
                            cbf[:].rearrange("b (k p) -> b k p", k=K)[:, :, :, None]
                            .broadcast_to([TB, K, P, E]),
                        )
                        nc.vector.reduce_sum(
                            s_sb[:].rearrange("b (k e) -> b k e", k=K),
                            tmpv.rearrange("b k p e -> b k e p"),
                            axis=mybir.AxisListType.X,
                        )
                        _squash(nc, rp, s_sb[:], v32[:], vbf[:])
                    if it < 2:
                        # agreement: bl += sum_e u*v
                        nc.vector.tensor_mul(
                            tmpv,
                            u1v,
                            vbf[:].rearrange("b (k e) -> b k e", k=K)[:, :, None, :]
                            .broadcast_to([TB, K, P, E]),
                        )
                        if it == 0:
                            nc.vector.reduce_sum(
                                bl[:].rearrange("b (k p) -> b k p", k=K),
                                tmpv,
                                axis=mybir.AxisListType.X,
                            )
                        else:
                            bld = rp.tile([TB, PAIRS], F32, tag="bld")
                            nc.vector.reduce_sum(
                                bld[:].rearrange("b (k p) -> b k p", k=K),
                                tmpv,
                                axis=mybir.AxisListType.X,
                            )
                            nc.vector.tensor_add(bl[:], bl[:], bld[:])

                nc.sync.dma_start(out_d[bt * TB:(bt + 1) * TB, :], v32[:])
    _split_excess_waits(nc)
    return nc


_NC_CACHE = {}


def kernel(x, conv_w, conv_b, weights, _trace=False):
    x = np.ascontiguousarray(np.asarray(x, dtype=np.float32))
    conv_w = np.asarray(conv_w, dtype=np.float32)
    conv_b = np.asarray(conv_b, dtype=np.float32)
    weights = np.asarray(weights, dtype=np.float32)

    # ---------------- host-side weight packing (tiny, O(weights))
    cwT = np.ascontiguousarray(conv_w.transpose(2, 0, 1).reshape(C, KO))
    w2blk = np.zeros((GROUPS, 128, 256), np.float32)
    w2s = np.zeros((GROUPS, 128, KE), np.float32)
    for t in range(GROUPS):
        for i in range(16):
            k, p = divmod(16 * t + i, P)
            w2blk[t, i * 8:(i + 1) * 8, i * 16:(i + 1) * 16] = weights[k, p]
            w2s[t, i * 8:(i + 1) * 8, k * 16:(k + 1) * 16] += 0.1 * weights[k, p]
    w2blk = w2blk.astype(ml_dtypes.bfloat16)
    w2s = w2s.astype(ml_dtypes.bfloat16)

    has_bias = bool(np.any(conv_b))
    extra = {}
    if has_bias:
        # biasU[k,p,e] = sum_d conv_b[k, (p*8+d)//36] * weights[k,p,d,e]
        g = np.arange(PAIRS * PD) % (P * PD)
        o_of = (g // HW).reshape(P, PD)  # o index for (p, d) within one k
        bU = np.einsum("kpd,kpde->kpe", conv_b[:, o_of], weights)
        bs1 = 0.1 * bU.sum(1)  # [K, E]
        extra["biasu"] = np.broadcast_to(
            bU.reshape(1, KPE).astype(ml_dtypes.bfloat16), (TB, KPE)
        ).copy()
        extra["biass1"] = np.broadcast_to(
            bs1.reshape(1, KE).astype(np.float32), (TB, KE)
        ).copy()

    # ---------------- shard + transpose x on host: [core][c, b, hw]
    xr = x.reshape(B_TOTAL, C, HW)
    in_maps = []
    for ci in range(N_CORES):
        xs = xr[ci * B_CORE:(ci + 1) * B_CORE]          # [256, 256, 36]
        xT = np.ascontiguousarray(xs.transpose(1, 0, 2)) # [c, b, hw]
        in_maps.append(
            {
                "xt": xT.reshape(2, 128, B_CORE * HW),
                "cwt": cwT,
                "w2blk": w2blk,
                "w2s": w2s,
                **extra,
            }
        )

    key = has_bias
    if key not in _NC_CACHE:
        _NC_CACHE[key] = _build_nc(has_bias)
    nc = _NC_CACHE[key]

    res = run_bass_kernel_spmd(
        nc, in_maps, core_ids=list(range(N_CORES)), trace=_trace
    )
    out = np.concatenate([r["out"] for r in res.results], axis=0)
    if _trace:
        kernel._last_result = res
    return out.reshape(B_TOTAL, K, E)


# revision 18
# speedup vs baseline: 1.2936x; 1.2936x over previous
"""DigitCapsules Trainium2 kernel (8-core batch-data-parallel).

Math (per sample b):
  caps[k,o,hw] = sum_c x[c,hw] conv_w[k,o,c] + conv_b[k,o]
  prim[k,p,d]  = caps[k, g//36, g%36],  g = p*8+d    (pure reindex)
  u[k,p,e]     = sum_d prim[k,p,d] W2[k,p,d,e]
  3 rounds of routing-by-agreement -> v[k,e]

Device mapping per core (256 samples = 2 tiles of 128 on partitions):
  * conv: 36x2 bf16 matmuls, stationary xT[c;b] per hw (contiguous, FWL),
    rhs conv_w[c;ko]; evac to caps[b; k*576+g] bf16
  * caps -> primT[(k,p,d); b] via 45 xbar DMA transposes (128-col chunks;
    chunk t == pair-group t because 576 = 72*8)
  * u-step: 45 block-diagonal matmuls primT_t @ W2blk_t -> psum; evacuated
    twice: u1[b; k,p,e] and u2[b; k,e,p] so both routing multiplies run in
    the DVE 2x bf16 mode. An extra cheap rhs (W2 summed over p, x0.1)
    accumulates s1 = iteration-1 weighted sum for free on the PE.
  * routing on DVE/ACT: softmax over k, weighted sums as 2x TT-mult +
    binary-tree reduction, squash with NR-refined sqrt.
"""

import sys

sys.path.insert(0, "/opt/trn_rl_repo")

import numpy as np
import ml_dtypes

import concourse.bass as bass
import concourse.mybir as mybir
from concourse import tile
from concourse.vector_clock import ScopedClock
from concourse.bass_utils import run_bass_kernel_spmd

# ---------------------------------------------------------------- constants
K, O, C, H, W = 10, 16, 256, 6, 6
HW = H * W                      # 36
PD, E, P = 8, 16, 72            # prim dim, out dim, prims per k
PAIRS = K * P                   # 720
GROUPS = PAIRS // 16            # 45 groups of 16 (k,p) pairs
KPE = PAIRS * E                 # 11520
KE = K * E                      # 160
KO = K * O                      # 160
B_TOTAL, N_CORES = 2048, 8
B_CORE = B_TOTAL // N_CORES     # 256
TB = 128                        # batch tile (partitions)
NT = B_CORE // TB               # 2
EPS = 1e-8

F32 = mybir.dt.float32
BF16 = mybir.dt.bfloat16


# ------------------------------------------------- tile drain-limit patch
# This walrus build accepts at most 1 sync wait on several instruction
# structs (CTRL drain, S3_LW ldweights, DMA pseudo). Tile piles one wait per
# outstanding sem lane onto single instructions; spill the extras onto
# same-engine NoOps placed immediately before (waits move earlier ->
# strictly more conservative, still correct).
def _patched_drain_and_barrier(self, tick_clock, wait_clock):
    nc = self.nc
    drain_inst = nc.sync.drain()
    wait_clock.add_sem_waits(
        drain_inst.ins, ScopedClock({None: tick_clock.global_clock})
    )
    ow = list(drain_inst.ins.sync_info.on_wait)
    if len(ow) > 1:
        drain_inst.ins.sync_info.on_wait = ow[:1]
        for w in ow[1:]:
            ni = nc.sync.nop()
            ni.ins.sync_info = mybir.SyncInfo(on_wait=[w], on_update=[])
    nc.all_engine_barrier()
    assert self.sems is not None
    popped = nc._tile_sem_poison_stack.pop()
    assert popped is self._sem_poison
    nc.clear_and_free_semaphores(list(self.sems.allocated().values()))
    nc.all_engine_barrier()


tile.TileContext._drain_and_barrier = _patched_drain_and_barrier

_MAXW = 1  # max sync waits this walrus accepts per instruction
_wsplit_n = [0]


def _split_excess_waits(nc):
    for f in nc.m.functions:
        for blk in f.blocks:
            insts = list(blk.instructions)
            out = []
            for inst in insts:
                si = inst.sync_info
                if si is not None and si.on_wait and len(si.on_wait) > _MAXW:
                    ow = list(si.on_wait)
                    extra, keep = ow[:-_MAXW], ow[-_MAXW:]
                    for i in range(0, len(extra), _MAXW):
                        _wsplit_n[0] += 1
                        out.append(
                            mybir.InstNoOp(
                                name=f"I-wsplit-{_wsplit_n[0]}",
                                engine=inst.engine,
                                sync_info=mybir.SyncInfo(
                                    on_wait=extra[i:i + _MAXW], on_update=[]
                                ),
                            )
                        )
                    si.on_wait = keep
                out.append(inst)
            if len(out) != len(insts):
                blk.instructions[:] = out


def _squash(nc, rp, s_ap, v_f32, v_bf16):
    """v = (|s|^2/(1+|s|^2)) * s / sqrt(|s|^2+eps); s_ap [128, K*E] f32."""
    t2 = rp.tile([TB, KE], F32, tag="sq_t2")
    nc.vector.tensor_mul(t2[:], s_ap, s_ap)
    ss = rp.tile([TB, K], F32, tag="sq_ss")
    nc.vector.reduce_sum(
        ss[:], t2[:].rearrange("b (k e) -> b k e", k=K), axis=mybir.AxisListType.X
    )
    ssq = rp.tile([TB, K], F32, tag="sq_ssq")
    nc.vector.tensor_scalar_add(ssq[:], ss[:], EPS)
    q = rp.tile([TB, K], F32, tag="sq_q")
    nc.scalar.sqrt(q[:], ssq[:])
    # one Newton step: q1 = 0.5*(q + ssq/q)  (guards ACT sqrt table error)
    rq = rp.tile([TB, K], F32, tag="sq_rq")
    nc.vector.reciprocal(rq[:], q[:])
    m1 = rp.tile([TB, K], F32, tag="sq_m1")
    nc.vector.tensor_mul(m1[:], ssq[:], rq[:])
    nc.vector.tensor_add(m1[:], m1[:], q[:])
    nc.vector.tensor_scalar_mul(m1[:], m1[:], 0.5)  # m1 = sqrt(ss+eps)
    a1 = rp.tile([TB, K], F32, tag="sq_a1")
    nc.vector.tensor_scalar_add(a1[:], ss[:], 1.0)
    nc.vector.tensor_mul(a1[:], a1[:], m1[:])       # (1+ss)*sqrt(ss+eps)
    rd = rp.tile([TB, K], F32, tag="sq_rd")
    nc.vector.reciprocal(rd[:], a1[:])
    f = rp.tile([TB, K], F32, tag="sq_f")
    nc.vector.tensor_mul(f[:], ss[:], rd[:])        # ss/((1+ss)*sqrt(..))
    fb = f[:, :, None].broadcast_to([TB, K, E])
    nc.vector.tensor_mul(
        v_f32.rearrange("b (k e) -> b k e", k=K),
        s_ap.rearrange("b (k e) -> b k e", k=K),
        fb,
    )
    nc.vector.tensor_copy(v_bf16, v_f32)


def _k_segments(pr0, np_):
    """Split pair range [pr0, pr0+np_) at k boundaries (multiples of P)."""
    segs = []
    a = pr0
    end = pr0 + np_
    while a < end:
        b = min(end, (a // P + 1) * P)
        segs.append((a, b))
        a = b
    return segs


def _build_nc(has_bias):
    nc = bass.Bass()
    x_d = nc.dram_tensor("xt", [NT, 2, 128, TB * HW], BF16, kind="ExternalInput")
    cw_d = nc.dram_tensor("cwt", [C, KO], BF16, kind="ExternalInput")
    w2_d = nc.dram_tensor("w2blk", [GROUPS, 128, 256], BF16, kind="ExternalInput")
    w2s_d = nc.dram_tensor("w2s", [GROUPS, 128, 32], BF16, kind="ExternalInput")
    if has_bias:
        bu_d = nc.dram_tensor("biasu", [TB, KPE], BF16, kind="ExternalInput")
        bu2_d = nc.dram_tensor("biasu2", [TB, KPE], BF16, kind="ExternalInput")
        bs_d = nc.dram_tensor("biass1", [TB, KE], F32, kind="ExternalInput")
    out_d = nc.dram_tensor("out", [B_CORE, KE], F32, kind="ExternalOutput")

    with tile.TileContext(nc) as tc:
        with (
            tc.tile_pool(name="consts", bufs=1) as consts,
            tc.tile_pool(name="xp", bufs=2) as xp,
            tc.tile_pool(name="big", bufs=1) as big,
            tc.tile_pool(name="rp", bufs=1) as rp,
            tc.tile_pool(name="ps_conv", bufs=3, space="PSUM") as ps_conv,
            tc.tile_pool(name="ps_u", bufs=2, space="PSUM") as ps_u,
            tc.tile_pool(name="ps_s1", bufs=1, space="PSUM") as ps_s1p,
        ):
            cw0 = consts.tile([128, KO], BF16)
            cw1 = consts.tile([128, KO], BF16)
            nc.sync.dma_start(cw0[:], cw_d[0:128, :])
            nc.sync.dma_start(cw1[:], cw_d[128:256, :])
            w2t = consts.tile([128, GROUPS * 256], BF16)
            nc.sync.dma_start(
                w2t[:].rearrange("c (t n) -> c t n", t=GROUPS),
                w2_d[:].rearrange("t c n -> c t n"),
            )
            w2st = consts.tile([128, GROUPS * 32], BF16)
            nc.sync.dma_start(
                w2st[:].rearrange("c (t n) -> c t n", t=GROUPS),
                w2s_d[:].rearrange("t c n -> c t n"),
            )
            if has_bias:
                but = consts.tile([TB, KPE], BF16)
                nc.sync.dma_start(but[:], bu_d[:])
                but2 = consts.tile([TB, KPE], BF16)
                nc.sync.dma_start(but2[:], bu2_d[:])
                bst = consts.tile([TB, KE], F32)
                nc.sync.dma_start(bst[:], bs_d[:])

            for bt in range(NT):
                # ------------------------------------------------ x load
                xt0 = xp.tile([128, TB * HW], BF16, tag="xt0")
                xt1 = xp.tile([128, TB * HW], BF16, tag="xt1")
                nc.sync.dma_start(xt0[:], x_d[bt, 0, :, :])
                nc.sync.dma_start(xt1[:], x_d[bt, 1, :, :])

                # ------------------------------------------------ conv
                caps = big.tile([TB, K * O * HW], BF16, tag="caps")
                capsv = caps[:].rearrange("b (k o hw) -> b k o hw", k=K, o=O)
                for hb in range(HW // 3):  # 12 psum banks of 3 hw each
                    pc = ps_conv.tile([TB, 3 * KO], F32, tag="pconv")
                    for j in range(3):
                        hw = hb * 3 + j
                        nc.tensor.matmul(
                            pc[:, j * KO:(j + 1) * KO],
                            xt0[:, hw * TB:(hw + 1) * TB],
                            cw0[:],
                            start=True, stop=False,
                        )
                        nc.tensor.matmul(
                            pc[:, j * KO:(j + 1) * KO],
                            xt1[:, hw * TB:(hw + 1) * TB],
                            cw1[:],
                            start=False, stop=True,
                        )
                    # evac [b; j,k,o] -> caps[b; k,o,hw=hb*3+j]
                    nc.scalar.copy(
                        capsv[:, :, :, hb * 3:hb * 3 + 3],
                        pc[:].rearrange("b (j k o) -> b k o j", j=3, k=K),
                    )

                # ----------------------- transpose to primT via xbar DMA
                primT = big.tile([128, GROUPS * 128], BF16, tag="primT")
                for t in range(GROUPS):
                    nc.sync.dma_start_transpose(
                        primT[:, t * 128:(t + 1) * 128],
                        caps[:, t * 128:(t + 1) * 128],
                    )

                # ------------------------------------------------ u-step
                u1 = big.tile([TB, KPE], BF16, tag="u1")
                u2 = big.tile([TB, KPE], BF16, tag="u2")
                u1v = u1[:].rearrange("b (k p e) -> b k p e", k=K, p=P)
                u2v = u2[:].rearrange("b (k e p) -> b k e p", k=K, e=E)
                ps1 = ps_s1p.tile([TB, 176], F32, tag="ps1")
                for ub in range(12):  # psum tiles of <=4 groups (2 banks)
                    ng = min(4, GROUPS - 4 * ub)
                    pu = ps_u.tile([TB, 4 * 256], F32, tag="pu")
                    for j in range(ng):
                        t = ub * 4 + j
                        nc.tensor.matmul(
                            pu[:, j * 256:(j + 1) * 256],
                            primT[:, t * 128:(t + 1) * 128],
                            w2t[:, t * 256:(t + 1) * 256],
                            start=True, stop=True,
                        )
                        k0 = (16 * t) // P
                        nc.tensor.matmul(
                            ps1[:, k0 * 16:k0 * 16 + 32],
                            primT[:, t * 128:(t + 1) * 128],
                            w2st[:, t * 32:(t + 1) * 32],
                            start=(t == 0), stop=(t == GROUPS - 1),
                            skip_group_check=True,
                        )
                    # u1 evac (contiguous); alternate engines for balance
                    eng = nc.vector.tensor_copy if ub % 2 == 0 else nc.scalar.copy
                    eng(u1[:, ub * 1024:ub * 1024 + ng * 256], pu[:, :ng * 256])
                    # u2 evac [b; k,e,p], split at k boundaries
                    pr0 = ub * 64
                    puv = pu[:].rearrange("b (pr e) -> b pr e", e=E)
                    for (a, b2) in _k_segments(pr0, ng * 16):
                        k = a // P
                        nc.scalar.copy(
                            u2v[:, k, :, a - k * P:b2 - k * P],
                            puv[:, a - pr0:b2 - pr0, :].rearrange("b pr e -> b e pr"),
                        )
                if has_bias:
                    nc.vector.tensor_add(u1[:], u1[:], but[:])
                    nc.vector.tensor_add(u2[:], u2[:], but2[:])

                # ------------------------------------------------ routing
                s_sb = rp.tile([TB, KE], F32, tag="s_sb")
                nc.scalar.copy(s_sb[:], ps1[:, :KE])
                if has_bias:
                    nc.vector.tensor_add(s_sb[:], s_sb[:], bst[:])
                v32 = rp.tile([TB, KE], F32, tag="v32")
                vbf = rp.tile([TB, KE], BF16, tag="vbf")
                _squash(nc, rp, s_sb[:], v32[:], vbf[:])

                bl = rp.tile([TB, PAIRS], F32, tag="bl")
                tmp = big.tile([TB, KPE], BF16, tag="tmp")
                tmpv = tmp[:].rearrange("b (k p e) -> b k p e", k=K, p=P)
                tmp2 = tmp  # same scratch tile; s-step and b-update are sequential
                t2v = tmp2[:].rearrange("b (ke p) -> b ke p", p=P)
                trh = big.tile([TB, KPE // 2], BF16, tag="trh")
                trhe = trh[:].rearrange("b (kp e) -> b kp e", e=8)
                trhp = trh[:].rearrange("b (ke p) -> b ke p", p=36)

                for it in range(3):
                    if it > 0:
                        # softmax over k -> c, then s = sum_p c*u  (u2 layout)
                        eb = rp.tile([TB, PAIRS], BF16, tag="eb")
                        nc.scalar.activation(
                            eb[:], bl[:], mybir.ActivationFunctionType.Exp
                        )
                        z = rp.tile([TB, P], F32, tag="z")
                        nc.vector.reduce_sum(
                            z[:],
                            eb[:].rearrange("b (k p) -> b p k", k=K),
                            axis=mybir.AxisListType.X,
                        )
                        rz = rp.tile([TB, P], F32, tag="rz")
                        nc.vector.reciprocal(rz[:], z[:])
                        rzb = rp.tile([TB, P], BF16, tag="rzb")
                        nc.vector.tensor_copy(rzb[:], rz[:])
                        cbf = rp.tile([TB, PAIRS], BF16, tag="cbf")
                        nc.vector.tensor_mul(
                            cbf[:].rearrange("b (k p) -> b k p", k=K),
                            eb[:].rearrange("b (k p) -> b k p", k=K),
                            rzb[:, None, :].broadcast_to([TB, K, P]),
                        )
                        # tmp2[b; (k,e), p] = u2 * c  (2x bf16)
                        nc.vector.tensor_mul(
                            tmp2[:].rearrange("b (k e p) -> b k e p", k=K, e=E),
                            u2v,
                            cbf[:].rearrange("b (k p) -> b k p", k=K)[:, :, None, :]
                            .broadcast_to([TB, K, E, P]),
                        )
                        # tree-reduce over p: 72 -> 36 -> 18 -> 9 -> X-reduce
                        nc.vector.tensor_add(
                            trhp[:, :, :], t2v[:, :, 0:36], t2v[:, :, 36:72]
                        )
                        nc.vector.tensor_add(
                            t2v[:, :, 0:18], trhp[:, :, 0:18], trhp[:, :, 18:36]
                        )
                        nc.vector.tensor_add(
                            trhp[:, :, 0:9], t2v[:, :, 0:9], t2v[:, :, 9:18]
                        )
                        nc.vector.reduce_sum(
                            s_sb[:].rearrange("b (k e) -> b k e", k=K),
                            trhp[:, :, 0:9],
                            axis=mybir.AxisListType.X,
                        )
                        _squash(nc, rp, s_sb[:], v32[:], vbf[:])
                    if it < 2:
                        # agreement: bl += sum_e u*v  (u1 layout, 2x bf16)
                        nc.vector.tensor_mul(
                            tmpv,
                            u1v,
                            vbf[:].rearrange("b (k e) -> b k e", k=K)[:, :, None, :]
                            .broadcast_to([TB, K, P, E]),
                        )
                        tv = tmp[:].rearrange("b (kp e) -> b kp e", e=E)
                        nc.vector.tensor_add(
                            trhe[:, :, :], tv[:, :, 0:8], tv[:, :, 8:16]
                        )
                        nc.vector.tensor_add(
                            tv[:, :, 0:4], trhe[:, :, 0:4], trhe[:, :, 4:8]
                        )
                        nc.vector.tensor_add(
                            trhe[:, :, 0:2], tv[:, :, 0:2], tv[:, :, 2:4]
                        )
                        if it == 0:
                            nc.vector.tensor_add(
                                bl[:], trhe[:, :, 0], trhe[:, :, 1]
                            )
                        else:
                            bld = rp.tile([TB, PAIRS], F32, tag="bld")
                            nc.vector.tensor_add(bld[:], trhe[:, :, 0], trhe[:, :, 1])
                            nc.vector.tensor_add(bl[:], bl[:], bld[:])

                nc.sync.dma_start(out_d[bt * TB:(bt + 1) * TB, :], v32[:])
    _split_excess_waits(nc)
    return nc


_NC_CACHE = {}


def kernel(x, conv_w, conv_b, weights, _trace=False):
    x = np.asarray(x, dtype=np.float32)
    conv_w = np.asarray(conv_w, dtype=np.float32)
    conv_b = np.asarray(conv_b, dtype=np.float32)
    weights = np.asarray(weights, dtype=np.float32)

    # ---------------- host-side weight packing (tiny, O(weights))
    cwT = conv_w.transpose(2, 0, 1).reshape(C, KO).astype(ml_dtypes.bfloat16)
    w2blk = np.zeros((GROUPS, 128, 256), np.float32)
    w2s = np.zeros((GROUPS, 128, 32), np.float32)
    for t in range(GROUPS):
        k0 = (16 * t) // P
        for i in range(16):
            k, p = divmod(16 * t + i, P)
            w2blk[t, i * 8:(i + 1) * 8, i * 16:(i + 1) * 16] = weights[k, p]
            w2s[t, i * 8:(i + 1) * 8, (k - k0) * 16:(k - k0 + 1) * 16] += (
                0.1 * weights[k, p]
            )
    w2blk = w2blk.astype(ml_dtypes.bfloat16)
    w2s = w2s.astype(ml_dtypes.bfloat16)

    has_bias = bool(np.any(conv_b))
    extra = {}
    if has_bias:
        g = np.arange(P * PD)
        o_of = (g // HW).reshape(P, PD)
        bU = np.einsum("kpd,kpde->kpe", conv_b[:, o_of], weights)
        bs1 = 0.1 * bU.sum(1)
        extra["biasu"] = np.broadcast_to(
            bU.reshape(1, KPE).astype(ml_dtypes.bfloat16), (TB, KPE)
        ).copy()
        extra["biasu2"] = np.broadcast_to(
            bU.transpose(0, 2, 1).reshape(1, KPE).astype(ml_dtypes.bfloat16),
            (TB, KPE),
        ).copy()
        extra["biass1"] = np.broadcast_to(
            bs1.reshape(1, KE).astype(np.float32), (TB, KE)
        ).copy()

    # -------- shard + transpose x on host: [core][bt, chunk, c, hw, b]
    xb = x.reshape(B_TOTAL, C, HW).astype(ml_dtypes.bfloat16)
    in_maps = []
    for ci in range(N_CORES):
        xs = xb[ci * B_CORE:(ci + 1) * B_CORE]            # [256, 256, 36]
        xs = xs.reshape(NT, TB, 2, 128, HW)               # bt, b, chunk, c, hw
        xT = np.ascontiguousarray(xs.transpose(0, 2, 3, 4, 1))  # bt,chunk,c,hw,b
        in_maps.append(
            {
                "xt": xT.reshape(NT, 2, 128, TB * HW),
                "cwt": cwT,
                "w2blk": w2blk,
                "w2s": w2s,
                **extra,
            }
        )

    key = has_bias
    if key not in _NC_CACHE:
        _NC_CACHE[key] = _build_nc(has_bias)
    nc = _NC_CACHE[key]

    res = run_bass_kernel_spmd(
        nc, in_maps, core_ids=list(range(N_CORES)), trace=_trace
    )
    out = np.concatenate([r["out"] for r in res.results], axis=0)
    if _trace:
        kernel._last_result = res
    return out.reshape(B_TOTAL, K, E)
